# revision 1
# baseline (speedup 1.0000x reference)
"""MoE ConditionalFeedForward (SwiGLU top-2 of 8 experts) on 8 Trainium2 cores.

Strategy: expert-parallel. Core c owns expert c's weights. The host routes
tokens: all (token, slot) assignments are bucketed by expert; each core runs
the dense SwiGLU FFN for up to C=512 of its expert's tokens (one full-width
matmul block). The handful of assignments beyond 512 per expert ("spill",
~1% of work) is computed on the host. Only activated pairs are computed
(~4x fewer FLOPs than the dense reference).

Matmuls run in float32r (full fp32 data, reduced-precision multiply at full
PE rate; free dims must be even). Layouts are feature-major ("transposed")
end to end so the contraction dim always sits on SBUF partitions and no
on-device transposes are needed:
  phase 1: h1T/h3T[i, t] = sum_d w1T[d, i] * xT[d, t]   (lhsT=w1 chunk, rhs=x)
  fuse:    hT = silu(h1T) * h3T
  phase 2: outT[d, t]    = sum_i w2T[i, d] * hT[i, t]

Phase-2 accumulation alternates between two PSUM banks (kic parity, merged by
a DVE add) so back-to-back matmuls never chain on one bank. Weights stream on
the sync HWDGE queue in consumption order with 8KB partition lines; x/out
ride the scalar queue.
"""

import numpy as np

T, A = 2048, 2
E, I, D = 8, 4096, 2048
N_CORES = 8
KC = D // 128   # 16 contraction chunks of 128 over D
IC = I // 128   # 32 i-chunks of 128
DC = D // 128   # 16 output d-chunks of 128

TRACE = False          # set by test harness to capture an NTFF profile
LAST_EXEC_NS = None    # filled when TRACE is set
_CACHE = {}            # compiled program cache keyed by (C, blocks)


def _split_blocks(C):
    """Split C tokens into even-sized matmul free-dim blocks (<=512).

    fp32r needs even block sizes; blocks >=256 keep fp32r at full rate."""
    nb = max(1, -(-C // 512))
    base = 2 * (-(-C // (2 * nb)))
    blocks = []
    rem = C
    for _ in range(nb - 1):
        blocks.append(base)
        rem -= base
    blocks.append(rem)
    assert all(b > 0 and b % 2 == 0 for b in blocks) and sum(blocks) == C
    return blocks


def _build_program(C, blocks):
    import concourse.bass as bass
    import concourse.tile as tile
    from concourse import bacc, mybir

    f32 = mybir.dt.float32
    f32r = mybir.dt.float32r

    nc = bacc.Bacc("TRN2", target_bir_lowering=False, debug=False,
                   num_devices=N_CORES)
    x_ap = nc.dram_tensor("x", [KC, 128, C], f32r, kind="ExternalInput").ap()
    w1_ap = nc.dram_tensor("w1", [IC, 128, KC * 128], f32r, kind="ExternalInput").ap()
    w3_ap = nc.dram_tensor("w3", [IC, 128, KC * 128], f32r, kind="ExternalInput").ap()
    w2_ap = nc.dram_tensor("w2", [DC, 128, IC * 128], f32r, kind="ExternalInput").ap()
    o_ap = nc.dram_tensor("o", [D, C], f32, kind="ExternalOutput").ap()

    boff = np.cumsum([0] + blocks)[:-1]

    with tile.TileContext(nc) as tc:
        with tc.tile_pool(name="xpool", bufs=1) as xpool, \
             tc.tile_pool(name="hpool", bufs=1) as hpool, \
             tc.tile_pool(name="w13", bufs=3) as w13pool, \
             tc.tile_pool(name="w2p", bufs=2) as w2pool, \
             tc.tile_pool(name="act", bufs=2) as actpool, \
             tc.tile_pool(name="outp", bufs=2) as outpool:

            # hoist the first weight tiles so the first matmuls' DMA-semaphore
            # waits don't cover the whole x transfer
            w13_head = {}
            for ic0 in range(1):
                tw1h = w13pool.tile([128, KC * 128], f32r, tag="tw1",
                                    name=f"tw1_{ic0}")
                tw3h = w13pool.tile([128, KC * 128], f32r, tag="tw3",
                                    name=f"tw3_{ic0}")
                nc.sync.dma_start(tw1h[:], w1_ap[ic0])
                nc.sync.dma_start(tw3h[:], w3_ap[ic0])
                w13_head[ic0] = (tw1h, tw3h)

            # resident: all x chunks [128, C] (one tile per k-chunk) and hT
            xts = []
            for kc in range(KC):
                xkc = xpool.tile([128, C], f32r, name=f"xt_{kc}")
                nc.scalar.dma_start(xkc[:], x_ap[kc])
                xts.append(xkc)
            ht = hpool.tile([128, IC * C], f32r, name="ht")

            # ---- phase 1: hT = silu(w1T.T @ x) * (w3T.T @ x), per i-chunk ----
            with tc.tile_pool(name="ps1", bufs=2, space="PSUM") as ps1:
                for ic in range(IC):
                    if ic in w13_head:
                        tw1, tw3 = w13_head[ic]
                    else:
                        tw1 = w13pool.tile([128, KC * 128], f32r, tag="tw1",
                                           name=f"tw1_{ic}")
                        tw3 = w13pool.tile([128, KC * 128], f32r, tag="tw3",
                                           name=f"tw3_{ic}")
                        nc.sync.dma_start(tw1[:], w1_ap[ic])
                        nc.sync.dma_start(tw3[:], w3_ap[ic])
                    for g0 in range(0, len(blocks), 2):
                        grp = list(enumerate(blocks))[g0:g0 + 2]
                        p1 = [ps1.tile([128, bn], f32, tag=f"p1_{bi - g0}",
                                       name=f"p1_{ic}_{bi}")
                              for bi, bn in grp]
                        p3 = [ps1.tile([128, bn], f32, tag=f"p3_{bi - g0}",
                                       name=f"p3_{ic}_{bi}")
                              for bi, bn in grp]
                        for kc in range(KC):
                            wsl1 = tw1[:, kc * 128:(kc + 1) * 128]
                            wsl3 = tw3[:, kc * 128:(kc + 1) * 128]
                            st, sp = (kc == 0), (kc == KC - 1)
                            for gi, (bi, bn) in enumerate(grp):
                                xsl = xts[kc][:, boff[bi]: boff[bi] + bn]
                                nc.tensor.matmul(p1[gi][:], wsl1, xsl, start=st, stop=sp)
                            for gi, (bi, bn) in enumerate(grp):
                                xsl = xts[kc][:, boff[bi]: boff[bi] + bn]
                                nc.tensor.matmul(p3[gi][:], wsl3, xsl, start=st, stop=sp)
                        for gi, (bi, bn) in enumerate(grp):
                            s1 = actpool.tile([128, bn], f32, tag=f"s1_{bi - g0}",
                                              name=f"s1_{ic}_{bi}")
                            nc.scalar.activation(s1[:], p1[gi][:],
                                                 mybir.ActivationFunctionType.Silu)
                            hsl = ht[:, ic * C + boff[bi]: ic * C + boff[bi] + bn]
                            nc.vector.tensor_mul(hsl, s1[:], p3[gi][:])

            # ---- phase 2: outT = w2T.T @ hT, per d-chunk ----
            with tc.tile_pool(name="ps2", bufs=2, space="PSUM") as ps2:
                for dc in range(DC):
                    # stream w2 d-chunk in two halves (8KB lines, fine deps)
                    tw2a = w2pool.tile([128, (IC // 2) * 128], f32r, tag="tw2a",
                                       name=f"tw2a_{dc}")
                    tw2b = w2pool.tile([128, (IC // 2) * 128], f32r, tag="tw2b",
                                       name=f"tw2b_{dc}")
                    nc.sync.dma_start(tw2a[:], w2_ap[dc, :, :(IC // 2) * 128])
                    nc.sync.dma_start(tw2b[:], w2_ap[dc, :, (IC // 2) * 128:])
                    ot = outpool.tile([128, C], f32, tag="ot", name=f"ot_{dc}")
                    for g0 in range(0, len(blocks), 2):
                        grp = list(enumerate(blocks))[g0:g0 + 2]
                        po = {}
                        for gi, (bi, bn) in enumerate(grp):
                            for par in (0, 1):
                                po[(gi, par)] = ps2.tile(
                                    [128, bn], f32, tag=f"po_{bi - g0}_{par}",
                                    name=f"po_{dc}_{bi}_{par}")
                        for kic in range(IC):
                            half = tw2a if kic < IC // 2 else tw2b
                            j = kic % (IC // 2)
                            wsl = half[:, j * 128:(j + 1) * 128]
                            par = kic % 2
                            st, sp = (kic < 2), (kic >= IC - 2)
                            for gi, (bi, bn) in enumerate(grp):
                                hsl = ht[:, kic * C + boff[bi]: kic * C + boff[bi] + bn]
                                nc.tensor.matmul(po[(gi, par)][:], wsl, hsl,
                                                 start=st, stop=sp)
                        for gi, (bi, bn) in enumerate(grp):
                            osl = ot[:, boff[bi]:boff[bi] + bn]
                            nc.vector.tensor_copy(osl, po[(gi, 0)][:])
                            nc.vector.tensor_add(osl, osl, po[(gi, 1)][:])
                    nc.scalar.dma_start(o_ap[dc * 128:(dc + 1) * 128, :], ot[:])

    nc.compile()
    return nc


def _run_spmd(nc, in_maps):
    global LAST_EXEC_NS
    from concourse import bass_utils
    if TRACE:
        import sys, types
        try:
            from antenv.axon_hooks import get_axon_ntff_profile_hook  # noqa
        except ImportError:
            from trn_agent_boot.trn_boot import _ntff_profile_via_ctypes
            _hook = _ntff_profile_via_ctypes('/opt/axon/libaxon_pjrt.so')
            m = types.ModuleType("antenv.axon_hooks")
            m.get_axon_ntff_profile_hook = lambda: _hook
            sys.modules["antenv.axon_hooks"] = m
        bass_utils.upload_artifacts = lambda tmpdir: "local://" + tmpdir
    res = bass_utils.run_bass_kernel_spmd(
        nc, in_maps, core_ids=list(range(N_CORES)), trace=TRACE)
    if TRACE:
        LAST_EXEC_NS = res.exec_time_ns
    return res.results


def kernel(x, expert_indices, w1, w2, w3):
    x = np.asarray(x)
    ei = np.asarray(expert_indices)
    w1 = np.asarray(w1)
    w2 = np.asarray(w2)
    w3 = np.asarray(w3)

    # ---- host routing ----
    flat = ei.reshape(-1).astype(np.int64)          # assignment -> expert
    order = np.argsort(flat, kind="stable")         # assignments grouped by expert
    counts = np.bincount(flat, minlength=E)
    off = np.concatenate([[0], np.cumsum(counts)])
    C = int(counts.max())
    C += C % 2                                      # fp32r wants even free dims
    C = max(min(C, 512), 2)                         # cap: spill goes to host
    blocks = tuple(_split_blocks(C))

    key = (C, blocks)
    if key not in _CACHE:
        _CACHE[key] = _build_program(C, list(blocks))
    nc = _CACHE[key]

    # token row lists per expert (first C assignments), padded with token 0;
    # assignments beyond C ("spill", a handful of tokens) are computed on host
    tok = np.zeros((E, C), dtype=np.int64)
    ndev = np.minimum(counts, C)
    for e in range(E):
        rows = order[off[e]:off[e] + ndev[e]] // A
        tok[e, :ndev[e]] = rows

    in_maps = []
    for e in range(E):
        xg = x[tok[e]]                                    # [C, D]
        xT = np.ascontiguousarray(xg.T).reshape(KC, 128, C)
        # w1/w3 [I, D] -> [ic, j, kc, p] -> [ic, p, kc, j]
        w1p = np.ascontiguousarray(
            w1[e].reshape(IC, 128, KC, 128).transpose(0, 3, 2, 1)
        ).reshape(IC, 128, KC * 128)
        w3p = np.ascontiguousarray(
            w3[e].reshape(IC, 128, KC, 128).transpose(0, 3, 2, 1)
        ).reshape(IC, 128, KC * 128)
        # w2 [D, I] -> [dc, j, kic, p] -> [dc, p, kic, j]
        w2p = np.ascontiguousarray(
            w2[e].reshape(DC, 128, IC, 128).transpose(0, 3, 2, 1)
        ).reshape(DC, 128, IC * 128)
        in_maps.append({"x": xT, "w1": w1p, "w3": w3p, "w2": w2p})

    results = _run_spmd(nc, in_maps)

    # ---- host scatter + spill compute ----
    out_flat = np.empty((T * A, D), dtype=np.float32)
    for e in range(E):
        oT = results[e]["o"]                              # [D, C]
        o_e = oT.T                                        # [C, D]
        idx = order[off[e]:off[e] + ndev[e]]
        out_flat[idx] = o_e[:ndev[e]]
        if counts[e] > ndev[e]:
            sidx = order[off[e] + ndev[e]:off[e + 1]]
            xs = x[sidx // A]                             # [s, D]
            h1 = xs @ w1[e].T
            h3 = xs @ w3[e].T
            h = (h1 / (1.0 + np.exp(-h1))) * h3
            out_flat[sidx] = h @ w2[e].T
    return out_flat.reshape(T, A, D)



# revision 2
# speedup vs baseline: 1.0805x; 1.0805x over previous
"""MoE ConditionalFeedForward (SwiGLU top-2 of 8 experts) on 8 Trainium2 cores.

Strategy: expert-parallel. Core c owns expert c's weights. The host routes
tokens: (token, expert) assignments are DEDUPED (a token listing the same
expert in both top-2 slots is computed once) and bucketed by expert; each
core runs the dense SwiGLU FFN for up to C<=512 of its expert's unique
tokens. Assignments beyond 512 per expert (spill; empty for the reference
distribution after dedup) are computed on the host.

Everything on-device runs in bfloat16 (tolerance is 2e-2; bf16 end-to-end
measures ~5e-3 max-rel). bf16 halves HBM traffic vs fp32r, enables the
fast-weight-load path so LDWEIGHTS fully hides under matmuls, and halves
SBUF pressure. Matmul accumulation is fp32 in PSUM.

Layouts are feature-major ("transposed") end to end so the contraction dim
always sits on SBUF partitions and no on-device transposes are needed:
  phase 1: h1T/h3T[i, t] = sum_d w1T[d, i] * xT[d, t]   (lhsT=w1 chunk, rhs=x)
  fuse:    hT = silu(h1T) * h3T                          (bf16, per-ic tiles)
  phase 2: outT[d, t]    = sum_i w2T[i, d] * hT[i, t]

hT is 32 separate per-ic tiles so phase-2 matmuls depend only on the ic
they read; ps1/ps2 PSUM pools are both open (disjoint banks) so phase 2
can start while phase 1 drains. Phase-2 accumulation alternates between
two PSUM banks (kic parity, merged by a DVE copy+add). The last d-chunk
is split [C-128, 128] so the final copy+DMA tail is short. Weights stream
on the sync HWDGE queue in consumption order; x/out ride the scalar queue.
"""

import numpy as np
import ml_dtypes

T, A = 2048, 2
E, I, D = 8, 4096, 2048
N_CORES = 8
KC = D // 128   # 16 contraction chunks of 128 over D
IC = I // 128   # 32 i-chunks of 128
DC = D // 128   # 16 output d-chunks of 128

TRACE = False          # set by test harness to capture an NTFF profile
LAST_EXEC_NS = None    # filled when TRACE is set
_CACHE = {}            # compiled program cache keyed by C


def _build_program(C):
    import concourse.bass as bass
    import concourse.tile as tile
    from concourse import bacc, mybir

    f32 = mybir.dt.float32
    bf16 = mybir.dt.bfloat16

    nc = bacc.Bacc("TRN2", target_bir_lowering=False, debug=False,
                   num_devices=N_CORES)
    x_ap = nc.dram_tensor("x", [KC, 128, C], bf16, kind="ExternalInput").ap()
    w1_ap = nc.dram_tensor("w1", [IC, 128, KC * 128], bf16, kind="ExternalInput").ap()
    w3_ap = nc.dram_tensor("w3", [IC, 128, KC * 128], bf16, kind="ExternalInput").ap()
    w2_ap = nc.dram_tensor("w2", [DC, 128, IC * 128], bf16, kind="ExternalInput").ap()
    o_ap = nc.dram_tensor("o", [D, C], bf16, kind="ExternalOutput").ap()

    with tile.TileContext(nc) as tc:
        with tc.tile_pool(name="xpool", bufs=1) as xpool, \
             tc.tile_pool(name="hpool", bufs=1) as hpool, \
             tc.tile_pool(name="w13", bufs=4) as w13pool, \
             tc.tile_pool(name="w2p", bufs=2) as w2pool, \
             tc.tile_pool(name="act", bufs=2) as actpool, \
             tc.tile_pool(name="outp", bufs=2) as outpool, \
             tc.tile_pool(name="ps1", bufs=2, space="PSUM") as ps1, \
             tc.tile_pool(name="ps2", bufs=2, space="PSUM") as ps2:

            # hoist the first weight tiles so the first matmuls' DMA-semaphore
            # waits don't cover the whole x transfer
            w13_head = {}
            for ic0 in range(1):
                tw1h = w13pool.tile([128, KC * 128], bf16, tag="tw1",
                                    name=f"tw1_{ic0}")
                tw3h = w13pool.tile([128, KC * 128], bf16, tag="tw3",
                                    name=f"tw3_{ic0}")
                nc.sync.dma_start(tw1h[:], w1_ap[ic0])
                nc.sync.dma_start(tw3h[:], w3_ap[ic0])
                w13_head[ic0] = (tw1h, tw3h)

            # resident: all x chunks [128, C] (one tile per k-chunk); hT as
            # one tile per i-chunk so phase-2 deps are per-ic, not whole-h
            xts = []
            for kc in range(KC):
                xkc = xpool.tile([128, C], bf16, name=f"xt_{kc}")
                nc.scalar.dma_start(xkc[:], x_ap[kc])
                xts.append(xkc)
            hts = [hpool.tile([128, C], bf16, name=f"ht_{ic}")
                   for ic in range(IC)]

            # ---- phase 1: hT = silu(w1T.T @ x) * (w3T.T @ x), per i-chunk ----
            for ic in range(IC):
                if ic in w13_head:
                    tw1, tw3 = w13_head[ic]
                else:
                    tw1 = w13pool.tile([128, KC * 128], bf16, tag="tw1",
                                       name=f"tw1_{ic}")
                    tw3 = w13pool.tile([128, KC * 128], bf16, tag="tw3",
                                       name=f"tw3_{ic}")
                    nc.sync.dma_start(tw1[:], w1_ap[ic])
                    nc.sync.dma_start(tw3[:], w3_ap[ic])
                p1 = ps1.tile([128, C], f32, tag="p1", name=f"p1_{ic}")
                p3 = ps1.tile([128, C], f32, tag="p3", name=f"p3_{ic}")
                for kc in range(KC):
                    wsl1 = tw1[:, kc * 128:(kc + 1) * 128]
                    wsl3 = tw3[:, kc * 128:(kc + 1) * 128]
                    st, sp = (kc == 0), (kc == KC - 1)
                    nc.tensor.matmul(p1[:], wsl1, xts[kc][:], start=st, stop=sp)
                    nc.tensor.matmul(p3[:], wsl3, xts[kc][:], start=st, stop=sp)
                s1 = actpool.tile([128, C], f32, tag="s1", name=f"s1_{ic}")
                nc.scalar.activation(s1[:], p1[:],
                                     mybir.ActivationFunctionType.Silu)
                nc.vector.tensor_mul(hts[ic][:], s1[:], p3[:])

            # ---- phase 2: outT = w2T.T @ hT, per d-chunk ----
            for dc in range(DC):
                # stream w2 d-chunk in two halves (fine-grained deps)
                tw2a = w2pool.tile([128, (IC // 2) * 128], bf16, tag="tw2a",
                                   name=f"tw2a_{dc}")
                tw2b = w2pool.tile([128, (IC // 2) * 128], bf16, tag="tw2b",
                                   name=f"tw2b_{dc}")
                nc.sync.dma_start(tw2a[:], w2_ap[dc, :, :(IC // 2) * 128])
                nc.sync.dma_start(tw2b[:], w2_ap[dc, :, (IC // 2) * 128:])
                ot = outpool.tile([128, C], bf16, tag="ot", name=f"ot_{dc}")
                # last d-chunk in two token-blocks so the final serial
                # copy+DMA tail after the last matmul is short
                blocks = [(0, C)] if dc < DC - 1 else [(0, C - 128), (C - 128, 128)]
                for bo, bn in blocks:
                    po = {}
                    for par in (0, 1):
                        po[par] = ps2.tile([128, bn], f32, tag=f"po_{par}",
                                           name=f"po_{dc}_{bo}_{par}")
                    for kic in range(IC):
                        half = tw2a if kic < IC // 2 else tw2b
                        j = kic % (IC // 2)
                        wsl = half[:, j * 128:(j + 1) * 128]
                        par = kic % 2
                        st, sp = (kic < 2), (kic >= IC - 2)
                        nc.tensor.matmul(po[par][:], wsl,
                                         hts[kic][:, bo:bo + bn],
                                         start=st, stop=sp)
                    osl = ot[:, bo:bo + bn]
                    nc.vector.tensor_copy(osl, po[0][:])
                    nc.vector.tensor_add(osl, osl, po[1][:])
                    nc.scalar.dma_start(
                        o_ap[dc * 128:(dc + 1) * 128, bo:bo + bn], osl)

    nc.compile()
    return nc


def _run_spmd(nc, in_maps):
    global LAST_EXEC_NS
    from concourse import bass_utils
    if TRACE:
        import sys, types
        try:
            from antenv.axon_hooks import get_axon_ntff_profile_hook  # noqa
        except ImportError:
            from trn_agent_boot.trn_boot import _ntff_profile_via_ctypes
            _hook = _ntff_profile_via_ctypes('/opt/axon/libaxon_pjrt.so')
            m = types.ModuleType("antenv.axon_hooks")
            m.get_axon_ntff_profile_hook = lambda: _hook
            sys.modules["antenv.axon_hooks"] = m
        bass_utils.upload_artifacts = lambda tmpdir: "local://" + tmpdir
    res = bass_utils.run_bass_kernel_spmd(
        nc, in_maps, core_ids=list(range(N_CORES)), trace=TRACE)
    if TRACE:
        LAST_EXEC_NS = res.exec_time_ns
    return res.results


def kernel(x, expert_indices, w1, w2, w3):
    x = np.asarray(x)
    ei = np.asarray(expert_indices)
    w1 = np.asarray(w1)
    w2 = np.asarray(w2)
    w3 = np.asarray(w3)

    # ---- host routing (dedup (token, expert) pairs) ----
    flat = ei.reshape(-1).astype(np.int64)          # assignment -> expert
    flat_tok = np.arange(T * A, dtype=np.int64) // A
    keys = flat * T + flat_tok                      # (expert, token) key
    uk = np.unique(keys)                            # sorted unique pairs
    ue, ut = uk // T, uk % T
    counts = np.bincount(ue, minlength=E)
    off = np.concatenate([[0], np.cumsum(counts)])
    C = int(counts.max())
    C += C % 2
    C = max(min(C, 512), 2)                         # cap: spill goes to host

    if C not in _CACHE:
        _CACHE[C] = _build_program(C)
    nc = _CACHE[C]

    # unique token row lists per expert (first C), padded with token 0
    tok = np.zeros((E, C), dtype=np.int64)
    ndev = np.minimum(counts, C)
    for e in range(E):
        tok[e, :ndev[e]] = ut[off[e]:off[e] + ndev[e]]

    bf = ml_dtypes.bfloat16
    w1b = w1.astype(bf)
    w2b = w2.astype(bf)
    w3b = w3.astype(bf)
    in_maps = []
    for e in range(E):
        xg = x[tok[e]]                                    # [C, D]
        xT = np.ascontiguousarray(xg.T.astype(bf)).reshape(KC, 128, C)
        # w1/w3 [I, D] -> [ic, j, kc, p] -> [ic, p, kc, j]
        w1p = np.ascontiguousarray(
            w1b[e].reshape(IC, 128, KC, 128).transpose(0, 3, 2, 1)
        ).reshape(IC, 128, KC * 128)
        w3p = np.ascontiguousarray(
            w3b[e].reshape(IC, 128, KC, 128).transpose(0, 3, 2, 1)
        ).reshape(IC, 128, KC * 128)
        # w2 [D, I] -> [dc, j, kic, p] -> [dc, p, kic, j]
        w2p = np.ascontiguousarray(
            w2b[e].reshape(DC, 128, IC, 128).transpose(0, 3, 2, 1)
        ).reshape(DC, 128, IC * 128)
        in_maps.append({"x": xT, "w1": w1p, "w3": w3p, "w2": w2p})

    results = _run_spmd(nc, in_maps)

    # ---- host scatter (each assignment gathers its expert's row) ----
    R = np.stack([np.asarray(results[e]["o"]).astype(np.float32)
                  for e in range(E)])                 # [E, D, C]
    pos = np.searchsorted(uk, keys)                   # row within expert block
    row = pos - off[flat]
    on_dev = row < ndev[flat]
    out_flat = np.empty((T * A, D), dtype=np.float32)
    out_flat[on_dev] = R[flat[on_dev], :, row[on_dev]]

    # spill (unique pairs beyond C per expert): host compute
    if not np.all(on_dev):
        sp = np.nonzero(~on_dev)[0]
        for i in sp:
            e, t = flat[i], flat_tok[i]
            xs = x[t]
            h1 = xs @ w1[e].T
            h3 = xs @ w3[e].T
            h = (h1 / (1.0 + np.exp(-h1))) * h3
            out_flat[i] = h @ w2[e].T
    return out_flat.reshape(T, A, D)


# revision 5
# speedup vs baseline: 1.0850x; 1.0042x over previous
"""MoE ConditionalFeedForward (SwiGLU top-2 of 8 experts) on 8 Trainium2 cores.

Strategy: expert-parallel. Core c owns expert c's weights. The host routes
tokens: (token, expert) assignments are DEDUPED (a token listing the same
expert in both top-2 slots is computed once) and bucketed by expert; each
core runs the dense SwiGLU FFN for up to C<=512 of its expert's unique
tokens. Assignments beyond 512 per expert (spill; empty for the reference
distribution after dedup) are computed on the host.

Everything on-device runs in bfloat16 (tolerance is 2e-2; bf16 end-to-end
measures ~5e-3 max-rel). bf16 halves HBM traffic vs fp32r, enables the
fast-weight-load path so LDWEIGHTS fully hides under matmuls, and halves
SBUF pressure. Matmul accumulation is fp32 in PSUM.

Layouts are feature-major ("transposed") end to end so the contraction dim
always sits on SBUF partitions and no on-device transposes are needed:
  phase 1: h1T/h3T[i, t] = sum_d w1T[d, i] * xT[d, t]   (lhsT=w1 chunk, rhs=x)
  fuse:    hT = silu(h1T) * h3T                          (bf16, per-ic tiles)
  phase 2: outT[d, t]    = sum_i w2T[i, d] * hT[i, t]

hT is 32 separate per-ic tiles so phase-2 matmuls depend only on the ic
they read; ps1/ps2 PSUM pools are both open (disjoint banks) so phase 2
can start while phase 1 drains. Phase-2 accumulation alternates between
two PSUM banks (kic parity, merged by a DVE copy+add). The last d-chunk
is split [C-128, 128] so the final copy+DMA tail is short.

A single HWDGE queue sustains only ~150 GB/s, which makes one 48MB weight
stream co-critical with the ~333us of matmuls; so weights ride TWO queues
(sync: w1 + w2 low halves; gpsimd: w3 + w2 high halves, ~24MB each). The
first i-chunk's weights are split into halves so the first matmul's DMA
wait is short. x is packed by the host into 4 big-line chunks on the
scalar queue (one dispatch per 4 k-chunks); out rides the scalar queue.
"""

import numpy as np
import ml_dtypes

T, A = 2048, 2
E, I, D = 8, 4096, 2048
N_CORES = 8
KC = D // 128   # 16 contraction chunks of 128 over D
IC = I // 128   # 32 i-chunks of 128
DC = D // 128   # 16 output d-chunks of 128

TRACE = False          # set by test harness to capture an NTFF profile
LAST_EXEC_NS = None    # filled when TRACE is set
_CACHE = {}            # compiled program cache keyed by C


def _build_program(C):
    import concourse.bass as bass
    import concourse.tile as tile
    from concourse import bacc, mybir

    f32 = mybir.dt.float32
    bf16 = mybir.dt.bfloat16

    nc = bacc.Bacc("TRN2", target_bir_lowering=False, debug=False,
                   num_devices=N_CORES)
    XG = 4              # x k-chunks per packed DMA
    x_ap = nc.dram_tensor("x", [KC // XG, 128, XG * C], bf16,
                          kind="ExternalInput").ap()
    w1_ap = nc.dram_tensor("w1", [IC, 128, KC * 128], bf16, kind="ExternalInput").ap()
    w3_ap = nc.dram_tensor("w3", [IC, 128, KC * 128], bf16, kind="ExternalInput").ap()
    w2_ap = nc.dram_tensor("w2", [DC, 128, IC * 128], bf16, kind="ExternalInput").ap()
    o_ap = nc.dram_tensor("o", [D, C], bf16, kind="ExternalOutput").ap()

    with tile.TileContext(nc) as tc:
        with tc.tile_pool(name="xpool", bufs=1) as xpool, \
             tc.tile_pool(name="hpool", bufs=1) as hpool, \
             tc.tile_pool(name="w13", bufs=6) as w13pool, \
             tc.tile_pool(name="w2p", bufs=2) as w2pool, \
             tc.tile_pool(name="act", bufs=2) as actpool, \
             tc.tile_pool(name="outp", bufs=2) as outpool, \
             tc.tile_pool(name="ps1", bufs=2, space="PSUM") as ps1, \
             tc.tile_pool(name="ps2", bufs=2, space="PSUM") as ps2:

            # hoist ic=0's weights, split into halves on separate queues so
            # the first matmuls' DMA-semaphore waits are short
            HK = KC // 2
            tw1ha = w13pool.tile([128, HK * 128], bf16, tag="tw1ha", name="tw1ha")
            tw3ha = w13pool.tile([128, HK * 128], bf16, tag="tw3ha", name="tw3ha")
            tw1hb = w13pool.tile([128, HK * 128], bf16, tag="tw1hb", name="tw1hb")
            tw3hb = w13pool.tile([128, HK * 128], bf16, tag="tw3hb", name="tw3hb")
            nc.sync.dma_start(tw1ha[:], w1_ap[0, :, :HK * 128])
            nc.gpsimd.dma_start(tw3ha[:], w3_ap[0, :, :HK * 128])
            nc.sync.dma_start(tw1hb[:], w1_ap[0, :, HK * 128:])
            nc.gpsimd.dma_start(tw3hb[:], w3_ap[0, :, HK * 128:])

            # resident: x in 4 packed chunks (4 k-chunks each, big DMA lines);
            # hT as one tile per i-chunk so phase-2 deps are per-ic
            xt4 = []
            for g in range(KC // XG):
                xg = xpool.tile([128, XG * C], bf16, name=f"xt4_{g}")
                nc.scalar.dma_start(xg[:], x_ap[g])
                xt4.append(xg)

            def xsl(kc):
                return xt4[kc // XG][:, (kc % XG) * C:(kc % XG + 1) * C]

            hts = [hpool.tile([128, C], bf16, name=f"ht_{ic}")
                   for ic in range(IC)]

            # ---- phase 1: hT = silu(w1T.T @ x) * (w3T.T @ x), per i-chunk ----
            for ic in range(IC):
                if ic == 0:
                    def wsl1(kc):
                        t = tw1ha if kc < HK else tw1hb
                        return t[:, (kc % HK) * 128:(kc % HK + 1) * 128]
                    def wsl3(kc):
                        t = tw3ha if kc < HK else tw3hb
                        return t[:, (kc % HK) * 128:(kc % HK + 1) * 128]
                else:
                    tw1 = w13pool.tile([128, KC * 128], bf16, tag="tw1",
                                       name=f"tw1_{ic}")
                    tw3 = w13pool.tile([128, KC * 128], bf16, tag="tw3",
                                       name=f"tw3_{ic}")
                    nc.sync.dma_start(tw1[:], w1_ap[ic])
                    nc.gpsimd.dma_start(tw3[:], w3_ap[ic])
                    def wsl1(kc, t=tw1):
                        return t[:, kc * 128:(kc + 1) * 128]
                    def wsl3(kc, t=tw3):
                        return t[:, kc * 128:(kc + 1) * 128]
                p1 = ps1.tile([128, C], f32, tag="p1", name=f"p1_{ic}")
                p3 = ps1.tile([128, C], f32, tag="p3", name=f"p3_{ic}")
                for kc in range(KC):
                    st, sp = (kc == 0), (kc == KC - 1)
                    nc.tensor.matmul(p1[:], wsl1(kc), xsl(kc), start=st, stop=sp)
                    nc.tensor.matmul(p3[:], wsl3(kc), xsl(kc), start=st, stop=sp)
                s1 = actpool.tile([128, C], f32, tag="s1", name=f"s1_{ic}")
                nc.scalar.activation(s1[:], p1[:],
                                     mybir.ActivationFunctionType.Silu)
                nc.vector.tensor_mul(hts[ic][:], s1[:], p3[:])

            # ---- phase 2: outT = w2T.T @ hT, per d-chunk ----
            for dc in range(DC):
                # stream w2 d-chunk in two halves on separate queues
                tw2a = w2pool.tile([128, (IC // 2) * 128], bf16, tag="tw2a",
                                   name=f"tw2a_{dc}")
                tw2b = w2pool.tile([128, (IC // 2) * 128], bf16, tag="tw2b",
                                   name=f"tw2b_{dc}")
                nc.sync.dma_start(tw2a[:], w2_ap[dc, :, :(IC // 2) * 128])
                nc.gpsimd.dma_start(tw2b[:], w2_ap[dc, :, (IC // 2) * 128:])
                ot = outpool.tile([128, C], bf16, tag="ot", name=f"ot_{dc}")
                # last d-chunk in two token-blocks so the final serial
                # copy+DMA tail after the last matmul is short
                blocks = [(0, C)] if dc < DC - 1 else [(0, C - 128), (C - 128, 128)]
                for bo, bn in blocks:
                    po = {}
                    for par in (0, 1):
                        po[par] = ps2.tile([128, bn], f32, tag=f"po_{par}",
                                           name=f"po_{dc}_{bo}_{par}")
                    for kic in range(IC):
                        half = tw2a if kic < IC // 2 else tw2b
                        j = kic % (IC // 2)
                        wsl = half[:, j * 128:(j + 1) * 128]
                        par = kic % 2
                        st, sp = (kic < 2), (kic >= IC - 2)
                        nc.tensor.matmul(po[par][:], wsl,
                                         hts[kic][:, bo:bo + bn],
                                         start=st, stop=sp)
                    osl = ot[:, bo:bo + bn]
                    nc.vector.tensor_copy(osl, po[0][:])
                    nc.vector.tensor_add(osl, osl, po[1][:])
                    nc.scalar.dma_start(
                        o_ap[dc * 128:(dc + 1) * 128, bo:bo + bn], osl)

    nc.compile()
    return nc


def _run_spmd(nc, in_maps):
    global LAST_EXEC_NS
    from concourse import bass_utils
    if TRACE:
        import sys, types
        try:
            from antenv.axon_hooks import get_axon_ntff_profile_hook  # noqa
        except ImportError:
            from trn_agent_boot.trn_boot import _ntff_profile_via_ctypes
            _hook = _ntff_profile_via_ctypes('/opt/axon/libaxon_pjrt.so')
            m = types.ModuleType("antenv.axon_hooks")
            m.get_axon_ntff_profile_hook = lambda: _hook
            sys.modules["antenv.axon_hooks"] = m
        bass_utils.upload_artifacts = lambda tmpdir: "local://" + tmpdir
    res = bass_utils.run_bass_kernel_spmd(
        nc, in_maps, core_ids=list(range(N_CORES)), trace=TRACE)
    if TRACE:
        LAST_EXEC_NS = res.exec_time_ns
    return res.results


def kernel(x, expert_indices, w1, w2, w3):
    x = np.asarray(x)
    ei = np.asarray(expert_indices)
    w1 = np.asarray(w1)
    w2 = np.asarray(w2)
    w3 = np.asarray(w3)

    # ---- host routing (dedup (token, expert) pairs) ----
    flat = ei.reshape(-1).astype(np.int64)          # assignment -> expert
    flat_tok = np.arange(T * A, dtype=np.int64) // A
    keys = flat * T + flat_tok                      # (expert, token) key
    uk = np.unique(keys)                            # sorted unique pairs
    ue, ut = uk // T, uk % T
    counts = np.bincount(ue, minlength=E)
    off = np.concatenate([[0], np.cumsum(counts)])
    C = int(counts.max())
    C += C % 2
    C = max(min(C, 512), 2)                         # cap: spill goes to host

    if C not in _CACHE:
        _CACHE[C] = _build_program(C)
    nc = _CACHE[C]

    # unique token row lists per expert (first C), padded with token 0
    tok = np.zeros((E, C), dtype=np.int64)
    ndev = np.minimum(counts, C)
    for e in range(E):
        tok[e, :ndev[e]] = ut[off[e]:off[e] + ndev[e]]

    bf = ml_dtypes.bfloat16
    w1b = w1.astype(bf)
    w2b = w2.astype(bf)
    w3b = w3.astype(bf)
    in_maps = []
    for e in range(E):
        xg = x[tok[e]]                                    # [C, D]
        # [KC, 128, C] -> packed [KC//4, 128, 4*C] (4 k-chunks per DMA)
        xT = np.ascontiguousarray(
            xg.T.astype(bf).reshape(KC // 4, 4, 128, C).transpose(0, 2, 1, 3)
        ).reshape(KC // 4, 128, 4 * C)
        # w1/w3 [I, D] -> [ic, j, kc, p] -> [ic, p, kc, j]
        w1p = np.ascontiguousarray(
            w1b[e].reshape(IC, 128, KC, 128).transpose(0, 3, 2, 1)
        ).reshape(IC, 128, KC * 128)
        w3p = np.ascontiguousarray(
            w3b[e].reshape(IC, 128, KC, 128).transpose(0, 3, 2, 1)
        ).reshape(IC, 128, KC * 128)
        # w2 [D, I] -> [dc, j, kic, p] -> [dc, p, kic, j]
        w2p = np.ascontiguousarray(
            w2b[e].reshape(DC, 128, IC, 128).transpose(0, 3, 2, 1)
        ).reshape(DC, 128, IC * 128)
        in_maps.append({"x": xT, "w1": w1p, "w3": w3p, "w2": w2p})

    results = _run_spmd(nc, in_maps)

    # ---- host scatter (each assignment gathers its expert's row) ----
    R = np.stack([np.asarray(results[e]["o"]).astype(np.float32)
                  for e in range(E)])                 # [E, D, C]
    pos = np.searchsorted(uk, keys)                   # row within expert block
    row = pos - off[flat]
    on_dev = row < ndev[flat]
    out_flat = np.empty((T * A, D), dtype=np.float32)
    out_flat[on_dev] = R[flat[on_dev], :, row[on_dev]]

    # spill (unique pairs beyond C per expert): host compute
    if not np.all(on_dev):
        sp = np.nonzero(~on_dev)[0]
        for i in sp:
            e, t = flat[i], flat_tok[i]
            xs = x[t]
            h1 = xs @ w1[e].T
            h3 = xs @ w3[e].T
            h = (h1 / (1.0 + np.exp(-h1))) * h3
            out_flat[i] = h @ w2[e].T
    return out_flat.reshape(T, A, D)


# revision 13
# speedup vs baseline: 1.0864x; 1.0013x over previous
"""MoE ConditionalFeedForward (SwiGLU top-2 of 8 experts) on 8 Trainium2 cores.

Strategy: expert-parallel. Core c owns expert c's weights. The host routes
tokens: (token, expert) assignments are DEDUPED (a token listing the same
expert in both top-2 slots is computed once) and bucketed by expert; each
core runs the dense SwiGLU FFN for up to C<=512 of its expert's unique
tokens. Assignments beyond 512 per expert (spill; empty for the reference
distribution after dedup) are computed on the host.

Everything on-device runs in bfloat16 (tolerance is 2e-2; bf16 end-to-end
measures ~5e-3 max-rel). bf16 halves HBM traffic vs fp32r, enables the
fast-weight-load path so LDWEIGHTS fully hides under matmuls, and halves
SBUF pressure. Matmul accumulation is fp32 in PSUM.

Layouts are feature-major ("transposed") end to end so the contraction dim
always sits on SBUF partitions and no on-device transposes are needed:
  phase 1: h1T/h3T[i, t] = sum_d w1T[d, i] * xT[d, t]   (lhsT=w1 chunk, rhs=x)
  fuse:    hT = silu(h1T) * h3T                          (bf16, per-ic tiles)
  phase 2: outT[d, t]    = sum_i w2T[i, d] * hT[i, t]

hT is 32 separate per-ic tiles so phase-2 matmuls depend only on the ic
they read; ps1/ps2 PSUM pools are both open (disjoint banks) so phase 2
can start while phase 1 drains. Phase-2 accumulation alternates between
two PSUM banks (kic parity, merged by a DVE copy+add). The last d-chunk
is split [C-128, 128] so the final copy+DMA tail is short.

A single HWDGE queue sustains only ~150 GB/s and phase 1 consumes 2MB of
weights per 6.8us i-chunk (~294 GB/s), so phase-1 weights ride FOUR queues
(w1 halves on sync+vector, w3 halves on gpsimd+scalar, ~74 GB/s each);
phase 2 streams w2 halves on sync+gpsimd. x is packed by the host into 5
chunks interleaved across queues so ic0's k-chunks land just in time. out
rides the scalar queue (idle during phase 2). Before the first real
matmul, ~36 dummy N=128 matmuls on a memset scratch tile warm the PE's
HAM clock gate during the initial DMA wait, so real matmuls start at
2.4 GHz instead of ramping from 1.2.
"""

import numpy as np
import ml_dtypes

T, A = 2048, 2
E, I, D = 8, 4096, 2048
N_CORES = 8
KC = D // 128   # 16 contraction chunks of 128 over D
IC = I // 128   # 32 i-chunks of 128
DC = D // 128   # 16 output d-chunks of 128

TRACE = False          # set by test harness to capture an NTFF profile
LAST_EXEC_NS = None    # filled when TRACE is set
_CACHE = {}            # compiled program cache keyed by C


def _build_program(C):
    import concourse.bass as bass
    import concourse.tile as tile
    from concourse import bacc, mybir

    f32 = mybir.dt.float32
    bf16 = mybir.dt.bfloat16

    nc = bacc.Bacc("TRN2", target_bir_lowering=False, debug=False,
                   num_devices=N_CORES)
    HK = KC // 2        # k-chunks in a weight low-half tile
    QK = KC // 4        # k-chunks in a weight quarter tile
    # x ships in 4 uniform chunks of 4 k-chunks, spread across queues
    x4_ap = nc.dram_tensor("x4", [4, 128, 4 * C], bf16, kind="ExternalInput").ap()
    w1_ap = nc.dram_tensor("w1", [IC, 128, KC * 128], bf16, kind="ExternalInput").ap()
    w3_ap = nc.dram_tensor("w3", [IC, 128, KC * 128], bf16, kind="ExternalInput").ap()
    w2_ap = nc.dram_tensor("w2", [DC, 128, IC * 128], bf16, kind="ExternalInput").ap()
    o_ap = nc.dram_tensor("o", [D, C], bf16, kind="ExternalOutput").ap()

    with tile.TileContext(nc) as tc:
        with tc.tile_pool(name="xpool", bufs=1) as xpool, \
             tc.tile_pool(name="hpool", bufs=1) as hpool, \
             tc.tile_pool(name="w13", bufs=6) as w13pool, \
             tc.tile_pool(name="w2p", bufs=2) as w2pool, \
             tc.tile_pool(name="act", bufs=2) as actpool, \
             tc.tile_pool(name="outp", bufs=2) as outpool, \
             tc.tile_pool(name="ps1", bufs=2, space="PSUM") as ps1, \
             tc.tile_pool(name="ps2", bufs=2, space="PSUM") as ps2:

            # PE pre-warm: dummy matmuls on a memset scratch tile keep the
            # HAM clock gate busy during the initial DMA wait
            scr = xpool.tile([128, 128], bf16, name="warm_scr")
            nc.gpsimd.memset(scr[:], 0)
            pdum = ps1.tile([128, 128], f32, tag="p1", name="pdum")
            for _ in range(36):
                nc.tensor.matmul(pdum[:], scr[:], scr[:], start=True, stop=True)
            scr2 = actpool.tile([128, 128], f32, tag="dumout", name="warm_out")
            nc.vector.tensor_copy(scr2[:], pdum[:])

            def w13_tiles(ic):
                # per-ic, per-tensor: low half (kc 0-7) + two quarters
                tiles = {}
                for wname in ("1", "3"):
                    tiles[wname] = (
                        w13pool.tile([128, HK * 128], bf16, tag=f"tw{wname}l",
                                     name=f"tw{wname}l_{ic}"),
                        w13pool.tile([128, QK * 128], bf16, tag=f"tw{wname}h1",
                                     name=f"tw{wname}h1_{ic}"),
                        w13pool.tile([128, QK * 128], bf16, tag=f"tw{wname}h2",
                                     name=f"tw{wname}h2_{ic}"),
                    )
                return tiles

            def w13_dma(q3, tiles, w_ap, ic):
                ql, q1, q2 = q3
                ql.dma_start(tiles[0][:], w_ap[ic, :, :HK * 128])
                q1.dma_start(tiles[1][:], w_ap[ic, :, HK * 128:(HK + QK) * 128])
                q2.dma_start(tiles[2][:], w_ap[ic, :, (HK + QK) * 128:])

            # ic0 weights + x chunks interleaved so each queue delivers
            # ic0's operands roughly in consumption order
            xts = [xpool.tile([128, 4 * C], bf16, name=f"xc{g}")
                   for g in range(4)]
            ic0t = w13_tiles(0)
            nc.scalar.dma_start(xts[0][:], x4_ap[0])        # kc 0-3
            nc.sync.dma_start(ic0t["1"][0][:], w1_ap[0, :, :HK * 128])
            nc.gpsimd.dma_start(ic0t["3"][0][:], w3_ap[0, :, :HK * 128])
            nc.scalar.dma_start(xts[1][:], x4_ap[1])        # kc 4-7
            nc.gpsimd.dma_start(xts[2][:], x4_ap[2])        # kc 8-11
            nc.sync.dma_start(ic0t["1"][1][:],
                              w1_ap[0, :, HK * 128:(HK + QK) * 128])
            nc.gpsimd.dma_start(ic0t["3"][1][:],
                                w3_ap[0, :, HK * 128:(HK + QK) * 128])
            nc.sync.dma_start(ic0t["1"][2][:], w1_ap[0, :, (HK + QK) * 128:])
            nc.gpsimd.dma_start(ic0t["3"][2][:], w3_ap[0, :, (HK + QK) * 128:])
            nc.sync.dma_start(xts[3][:], x4_ap[3])          # kc 12-15

            def xsl(kc):
                return xts[kc // 4][:, (kc % 4) * C:(kc % 4 + 1) * C]

            hts = [hpool.tile([128, C], bf16, name=f"ht_{ic}")
                   for ic in range(IC)]

            # ---- phase 1: hT = silu(w1T.T @ x) * (w3T.T @ x), per i-chunk ----
            for ic in range(IC):
                if ic == 0:
                    tls = ic0t
                else:
                    tls = w13_tiles(ic)
                    w13_dma((nc.sync, nc.sync, nc.scalar), tls["1"], w1_ap, ic)
                    w13_dma((nc.gpsimd, nc.gpsimd, nc.scalar), tls["3"], w3_ap, ic)

                def _wsl(t3, kc):
                    if kc < HK:
                        return t3[0][:, kc * 128:(kc + 1) * 128]
                    j = kc - HK
                    return t3[1 + j // QK][:, (j % QK) * 128:(j % QK + 1) * 128]

                def wsl1(kc):
                    return _wsl(tls["1"], kc)

                def wsl3(kc):
                    return _wsl(tls["3"], kc)

                p1 = ps1.tile([128, C], f32, tag="p1", name=f"p1_{ic}")
                p3 = ps1.tile([128, C], f32, tag="p3", name=f"p3_{ic}")
                for kc in range(KC):
                    st, sp = (kc == 0), (kc == KC - 1)
                    nc.tensor.matmul(p1[:], wsl1(kc), xsl(kc), start=st, stop=sp)
                    nc.tensor.matmul(p3[:], wsl3(kc), xsl(kc), start=st, stop=sp)
                s1 = actpool.tile([128, C], f32, tag="s1", name=f"s1_{ic}")
                nc.scalar.activation(s1[:], p1[:],
                                     mybir.ActivationFunctionType.Silu)
                nc.vector.tensor_mul(hts[ic][:], s1[:], p3[:])

            # ---- phase 2: outT = w2T.T @ hT, per d-chunk ----
            for dc in range(DC):
                # stream w2 d-chunk in two halves on separate queues
                tw2a = w2pool.tile([128, (IC // 2) * 128], bf16, tag="tw2a",
                                   name=f"tw2a_{dc}")
                tw2b = w2pool.tile([128, (IC // 2) * 128], bf16, tag="tw2b",
                                   name=f"tw2b_{dc}")
                nc.sync.dma_start(tw2a[:], w2_ap[dc, :, :(IC // 2) * 128])
                nc.gpsimd.dma_start(tw2b[:], w2_ap[dc, :, (IC // 2) * 128:])
                ot = outpool.tile([128, C], bf16, tag="ot", name=f"ot_{dc}")
                # last d-chunk in two token-blocks so the final serial
                # copy+DMA tail after the last matmul is short
                blocks = [(0, C)] if dc < DC - 1 else [(0, C - 128), (C - 128, 128)]
                for bo, bn in blocks:
                    po = {}
                    for par in (0, 1):
                        po[par] = ps2.tile([128, bn], f32, tag=f"po_{par}",
                                           name=f"po_{dc}_{bo}_{par}")
                    for kic in range(IC):
                        half = tw2a if kic < IC // 2 else tw2b
                        j = kic % (IC // 2)
                        wsl = half[:, j * 128:(j + 1) * 128]
                        par = kic % 2
                        st, sp = (kic < 2), (kic >= IC - 2)
                        nc.tensor.matmul(po[par][:], wsl,
                                         hts[kic][:, bo:bo + bn],
                                         start=st, stop=sp)
                    osl = ot[:, bo:bo + bn]
                    nc.vector.tensor_copy(osl, po[0][:])
                    nc.vector.tensor_add(osl, osl, po[1][:])
                    nc.scalar.dma_start(
                        o_ap[dc * 128:(dc + 1) * 128, bo:bo + bn], osl)

    nc.compile()
    return nc


def _run_spmd(nc, in_maps):
    global LAST_EXEC_NS
    from concourse import bass_utils
    if TRACE:
        import sys, types
        try:
            from antenv.axon_hooks import get_axon_ntff_profile_hook  # noqa
        except ImportError:
            from trn_agent_boot.trn_boot import _ntff_profile_via_ctypes
            _hook = _ntff_profile_via_ctypes('/opt/axon/libaxon_pjrt.so')
            m = types.ModuleType("antenv.axon_hooks")
            m.get_axon_ntff_profile_hook = lambda: _hook
            sys.modules["antenv.axon_hooks"] = m
        bass_utils.upload_artifacts = lambda tmpdir: "local://" + tmpdir
    res = bass_utils.run_bass_kernel_spmd(
        nc, in_maps, core_ids=list(range(N_CORES)), trace=TRACE)
    if TRACE:
        LAST_EXEC_NS = res.exec_time_ns
    return res.results


def kernel(x, expert_indices, w1, w2, w3):
    x = np.asarray(x)
    ei = np.asarray(expert_indices)
    w1 = np.asarray(w1)
    w2 = np.asarray(w2)
    w3 = np.asarray(w3)

    # ---- host routing (dedup (token, expert) pairs) ----
    flat = ei.reshape(-1).astype(np.int64)          # assignment -> expert
    flat_tok = np.arange(T * A, dtype=np.int64) // A
    keys = flat * T + flat_tok                      # (expert, token) key
    uk = np.unique(keys)                            # sorted unique pairs
    ue, ut = uk // T, uk % T
    counts = np.bincount(ue, minlength=E)
    off = np.concatenate([[0], np.cumsum(counts)])
    C = int(counts.max())
    C += C % 2
    C = max(min(C, 512), 2)                         # cap: spill goes to host

    if C not in _CACHE:
        _CACHE[C] = _build_program(C)
    nc = _CACHE[C]

    # unique token row lists per expert (first C), padded with token 0
    tok = np.zeros((E, C), dtype=np.int64)
    ndev = np.minimum(counts, C)
    for e in range(E):
        tok[e, :ndev[e]] = ut[off[e]:off[e] + ndev[e]]

    bf = ml_dtypes.bfloat16
    w1b = w1.astype(bf)
    w2b = w2.astype(bf)
    w3b = w3.astype(bf)
    in_maps = []
    for e in range(E):
        xg = x[tok[e]]                                    # [C, D]
        # [KC, 128, C] -> 4 chunks of 4 k-chunks, each packed
        # [128, n*C] with k-chunk-major columns
        xT = xg.T.astype(bf).reshape(KC, 128, C)

        def pack(lo, n):
            return np.ascontiguousarray(
                xT[lo:lo + n].transpose(1, 0, 2)).reshape(128, n * C)
        x4 = np.stack([pack(4 * g, 4) for g in range(4)])  # [4, 128, 4C]
        # w1/w3 [I, D] -> [ic, j, kc, p] -> [ic, p, kc, j]
        w1p = np.ascontiguousarray(
            w1b[e].reshape(IC, 128, KC, 128).transpose(0, 3, 2, 1)
        ).reshape(IC, 128, KC * 128)
        w3p = np.ascontiguousarray(
            w3b[e].reshape(IC, 128, KC, 128).transpose(0, 3, 2, 1)
        ).reshape(IC, 128, KC * 128)
        # w2 [D, I] -> [dc, j, kic, p] -> [dc, p, kic, j]
        w2p = np.ascontiguousarray(
            w2b[e].reshape(DC, 128, IC, 128).transpose(0, 3, 2, 1)
        ).reshape(DC, 128, IC * 128)
        in_maps.append({"x4": x4, "w1": w1p, "w3": w3p, "w2": w2p})

    results = _run_spmd(nc, in_maps)

    # ---- host scatter (each assignment gathers its expert's row) ----
    R = np.stack([np.asarray(results[e]["o"]).astype(np.float32)
                  for e in range(E)])                 # [E, D, C]
    pos = np.searchsorted(uk, keys)                   # row within expert block
    row = pos - off[flat]
    on_dev = row < ndev[flat]
    out_flat = np.empty((T * A, D), dtype=np.float32)
    out_flat[on_dev] = R[flat[on_dev], :, row[on_dev]]

    # spill (unique pairs beyond C per expert): host compute
    if not np.all(on_dev):
        sp = np.nonzero(~on_dev)[0]
        for i in sp:
            e, t = flat[i], flat_tok[i]
            xs = x[t]
            h1 = xs @ w1[e].T
            h3 = xs @ w3[e].T
            h = (h1 / (1.0 + np.exp(-h1))) * h3
            out_flat[i] = h @ w2[e].T
    return out_flat.reshape(T, A, D)


# revision 15
# speedup vs baseline: 1.1013x; 1.0137x over previous
"""MoE ConditionalFeedForward (SwiGLU top-2 of 8 experts) on 8 Trainium2 cores.

Strategy: expert-parallel. Core c owns expert c's weights. The host routes
tokens: (token, expert) assignments are DEDUPED (a token listing the same
expert in both top-2 slots is computed once) and bucketed by expert; each
core runs the dense SwiGLU FFN for up to C<=512 of its expert's unique
tokens. Assignments beyond 512 per expert (spill; empty for the reference
distribution after dedup) are computed on the host.

Everything on-device runs in bfloat16 (tolerance is 2e-2; bf16 end-to-end
measures ~5e-3 max-rel). bf16 halves HBM traffic vs fp32r, enables the
fast-weight-load path so LDWEIGHTS fully hides under matmuls, and halves
SBUF pressure. Matmul accumulation is fp32 in PSUM.

Layouts are feature-major ("transposed") end to end so the contraction dim
always sits on SBUF partitions and no on-device transposes are needed:
  phase 1: h1T/h3T[i, t] = sum_d w1T[d, i] * xT[d, t]   (lhsT=w1 chunk, rhs=x)
  fuse:    hT = silu(h1T) * h3T                          (bf16, per-ic tiles)
  phase 2: outT[d, t]    = sum_i w2T[i, d] * hT[i, t]

hT is 32 separate per-ic tiles so phase-2 matmuls depend only on the ic
they read; ps1/ps2 PSUM pools are both open (disjoint banks) so phase 2
can start while phase 1 drains. Phase-2 accumulation alternates between
two PSUM banks (kic parity, merged by a DVE copy+add). The last d-chunk
is split [C-128, 128] so the final copy+DMA tail is short.

DMA model (measured): a queue sustains up to ~340 GB/s, but each engine
dispatches only ~1 descriptor per 0.65us and the fabric ramps over the
first ~10us — so the early window wants FEW, BIG transfers spread across
the three dispatching engines (sync, gpsimd, scalar). Weights stream as
full 1MB per-ic tiles (w1 on sync, w3 on gpsimd, ~147 GB/s each, 2.3x
headroom); ic0's tiles are split in halves and x's four 0.5MB chunks are
placed so every ic0 operand lands just in time (first matmul ~9.5us).
Phase 2 streams w2 halves on sync+gpsimd; out rides the scalar queue.
Before the first real matmul, dummy N=128 matmuls on a memset scratch
tile warm the PE's HAM clock gate during the initial DMA wait, so real
matmuls start at 2.4 GHz instead of ramping from 1.2.
"""

import numpy as np
import ml_dtypes

T, A = 2048, 2
E, I, D = 8, 4096, 2048
N_CORES = 8
KC = D // 128   # 16 contraction chunks of 128 over D
IC = I // 128   # 32 i-chunks of 128
DC = D // 128   # 16 output d-chunks of 128

TRACE = False          # set by test harness to capture an NTFF profile
LAST_EXEC_NS = None    # filled when TRACE is set
_CACHE = {}            # compiled program cache keyed by C


def _build_program(C):
    import concourse.bass as bass
    import concourse.tile as tile
    from concourse import bacc, mybir

    f32 = mybir.dt.float32
    bf16 = mybir.dt.bfloat16

    nc = bacc.Bacc("TRN2", target_bir_lowering=False, debug=False,
                   num_devices=N_CORES)
    HK = KC // 2        # k-chunks in a weight low-half tile
    QK = KC // 4        # k-chunks in a weight quarter tile
    # x ships in 4 uniform chunks of 4 k-chunks, spread across queues
    x4_ap = nc.dram_tensor("x4", [4, 128, 4 * C], bf16, kind="ExternalInput").ap()
    w1_ap = nc.dram_tensor("w1", [IC, 128, KC * 128], bf16, kind="ExternalInput").ap()
    w3_ap = nc.dram_tensor("w3", [IC, 128, KC * 128], bf16, kind="ExternalInput").ap()
    w2_ap = nc.dram_tensor("w2", [DC, 128, IC * 128], bf16, kind="ExternalInput").ap()
    o_ap = nc.dram_tensor("o", [D, C], bf16, kind="ExternalOutput").ap()

    with tile.TileContext(nc) as tc:
        with tc.tile_pool(name="xpool", bufs=1) as xpool, \
             tc.tile_pool(name="hpool", bufs=1) as hpool, \
             tc.tile_pool(name="w13", bufs=6) as w13pool, \
             tc.tile_pool(name="w2p", bufs=2) as w2pool, \
             tc.tile_pool(name="act", bufs=2) as actpool, \
             tc.tile_pool(name="outp", bufs=2) as outpool, \
             tc.tile_pool(name="ps1", bufs=2, space="PSUM") as ps1, \
             tc.tile_pool(name="ps2", bufs=2, space="PSUM") as ps2:

            # PE pre-warm: dummy matmuls on a memset scratch tile keep the
            # HAM clock gate busy during the initial DMA wait
            scr = xpool.tile([128, 128], bf16, name="warm_scr")
            nc.gpsimd.memset(scr[:], 0)
            pdum = ps1.tile([128, 128], f32, tag="p1", name="pdum")
            for _ in range(36):
                nc.tensor.matmul(pdum[:], scr[:], scr[:], start=True, stop=True)
            scr2 = actpool.tile([128, 128], f32, tag="dumout", name="warm_out")
            nc.vector.tensor_copy(scr2[:], pdum[:])

            # ic0's weight tiles split in halves; x in four 0.5MB chunks,
            # queue order chosen so every ic0 operand lands just in time
            xts = [xpool.tile([128, 4 * C], bf16, name=f"xc{g}")
                   for g in range(4)]
            t1l0 = w13pool.tile([128, HK * 128], bf16, tag="tw1", name="tw1l_0")
            t1h0 = w13pool.tile([128, HK * 128], bf16, tag="tw1", name="tw1h_0")
            t3l0 = w13pool.tile([128, HK * 128], bf16, tag="tw3", name="tw3l_0")
            t3h0 = w13pool.tile([128, HK * 128], bf16, tag="tw3", name="tw3h_0")
            nc.scalar.dma_start(xts[0][:], x4_ap[0])        # kc 0-3
            nc.sync.dma_start(t1l0[:], w1_ap[0, :, :HK * 128])
            nc.gpsimd.dma_start(t3l0[:], w3_ap[0, :, :HK * 128])
            nc.scalar.dma_start(xts[1][:], x4_ap[1])        # kc 4-7
            nc.sync.dma_start(t1h0[:], w1_ap[0, :, HK * 128:])
            nc.gpsimd.dma_start(t3h0[:], w3_ap[0, :, HK * 128:])
            nc.sync.dma_start(xts[2][:], x4_ap[2])          # kc 8-11
            nc.scalar.dma_start(xts[3][:], x4_ap[3])        # kc 12-15

            def xsl(kc):
                return xts[kc // 4][:, (kc % 4) * C:(kc % 4 + 1) * C]

            hts = [hpool.tile([128, C], bf16, name=f"ht_{ic}")
                   for ic in range(IC)]

            # ---- phase 1: hT = silu(w1T.T @ x) * (w3T.T @ x), per i-chunk ----
            for ic in range(IC):
                if ic == 0:
                    def wsl1(kc):
                        t = t1l0 if kc < HK else t1h0
                        return t[:, (kc % HK) * 128:(kc % HK + 1) * 128]

                    def wsl3(kc):
                        t = t3l0 if kc < HK else t3h0
                        return t[:, (kc % HK) * 128:(kc % HK + 1) * 128]
                else:
                    tw1 = w13pool.tile([128, KC * 128], bf16, tag="tw1",
                                       name=f"tw1_{ic}")
                    tw3 = w13pool.tile([128, KC * 128], bf16, tag="tw3",
                                       name=f"tw3_{ic}")
                    nc.sync.dma_start(tw1[:], w1_ap[ic])
                    nc.gpsimd.dma_start(tw3[:], w3_ap[ic])

                    def wsl1(kc, t=tw1):
                        return t[:, kc * 128:(kc + 1) * 128]

                    def wsl3(kc, t=tw3):
                        return t[:, kc * 128:(kc + 1) * 128]

                p1 = ps1.tile([128, C], f32, tag="p1", name=f"p1_{ic}")
                p3 = ps1.tile([128, C], f32, tag="p3", name=f"p3_{ic}")
                for kc in range(KC):
                    st, sp = (kc == 0), (kc == KC - 1)
                    nc.tensor.matmul(p1[:], wsl1(kc), xsl(kc), start=st, stop=sp)
                    nc.tensor.matmul(p3[:], wsl3(kc), xsl(kc), start=st, stop=sp)
                s1 = actpool.tile([128, C], f32, tag="s1", name=f"s1_{ic}")
                nc.scalar.activation(s1[:], p1[:],
                                     mybir.ActivationFunctionType.Silu)
                nc.vector.tensor_mul(hts[ic][:], s1[:], p3[:])

            # ---- phase 2: outT = w2T.T @ hT, per d-chunk ----
            for dc in range(DC):
                # stream w2 d-chunk in two halves on separate queues
                tw2a = w2pool.tile([128, (IC // 2) * 128], bf16, tag="tw2a",
                                   name=f"tw2a_{dc}")
                tw2b = w2pool.tile([128, (IC // 2) * 128], bf16, tag="tw2b",
                                   name=f"tw2b_{dc}")
                nc.sync.dma_start(tw2a[:], w2_ap[dc, :, :(IC // 2) * 128])
                nc.gpsimd.dma_start(tw2b[:], w2_ap[dc, :, (IC // 2) * 128:])
                ot = outpool.tile([128, C], bf16, tag="ot", name=f"ot_{dc}")
                # last d-chunk in two token-blocks so the final serial
                # copy+DMA tail after the last matmul is short
                blocks = [(0, C)] if dc < DC - 1 else [(0, C - 128), (C - 128, 128)]
                for bo, bn in blocks:
                    po = {}
                    for par in (0, 1):
                        po[par] = ps2.tile([128, bn], f32, tag=f"po_{par}",
                                           name=f"po_{dc}_{bo}_{par}")
                    for kic in range(IC):
                        half = tw2a if kic < IC // 2 else tw2b
                        j = kic % (IC // 2)
                        wsl = half[:, j * 128:(j + 1) * 128]
                        par = kic % 2
                        st, sp = (kic < 2), (kic >= IC - 2)
                        nc.tensor.matmul(po[par][:], wsl,
                                         hts[kic][:, bo:bo + bn],
                                         start=st, stop=sp)
                    osl = ot[:, bo:bo + bn]
                    nc.vector.tensor_copy(osl, po[0][:])
                    nc.vector.tensor_add(osl, osl, po[1][:])
                    nc.scalar.dma_start(
                        o_ap[dc * 128:(dc + 1) * 128, bo:bo + bn], osl)

    nc.compile()
    return nc


def _run_spmd(nc, in_maps):
    global LAST_EXEC_NS
    from concourse import bass_utils
    if TRACE:
        import sys, types
        try:
            from antenv.axon_hooks import get_axon_ntff_profile_hook  # noqa
        except ImportError:
            from trn_agent_boot.trn_boot import _ntff_profile_via_ctypes
            _hook = _ntff_profile_via_ctypes('/opt/axon/libaxon_pjrt.so')
            m = types.ModuleType("antenv.axon_hooks")
            m.get_axon_ntff_profile_hook = lambda: _hook
            sys.modules["antenv.axon_hooks"] = m
        bass_utils.upload_artifacts = lambda tmpdir: "local://" + tmpdir
    res = bass_utils.run_bass_kernel_spmd(
        nc, in_maps, core_ids=list(range(N_CORES)), trace=TRACE)
    if TRACE:
        LAST_EXEC_NS = res.exec_time_ns
    return res.results


def kernel(x, expert_indices, w1, w2, w3):
    x = np.asarray(x)
    ei = np.asarray(expert_indices)
    w1 = np.asarray(w1)
    w2 = np.asarray(w2)
    w3 = np.asarray(w3)

    # ---- host routing (dedup (token, expert) pairs) ----
    flat = ei.reshape(-1).astype(np.int64)          # assignment -> expert
    flat_tok = np.arange(T * A, dtype=np.int64) // A
    keys = flat * T + flat_tok                      # (expert, token) key
    uk = np.unique(keys)                            # sorted unique pairs
    ue, ut = uk // T, uk % T
    counts = np.bincount(ue, minlength=E)
    off = np.concatenate([[0], np.cumsum(counts)])
    C = int(counts.max())
    C += C % 2
    C = max(min(C, 512), 2)                         # cap: spill goes to host

    if C not in _CACHE:
        _CACHE[C] = _build_program(C)
    nc = _CACHE[C]

    # unique token row lists per expert (first C), padded with token 0
    tok = np.zeros((E, C), dtype=np.int64)
    ndev = np.minimum(counts, C)
    for e in range(E):
        tok[e, :ndev[e]] = ut[off[e]:off[e] + ndev[e]]

    bf = ml_dtypes.bfloat16
    w1b = w1.astype(bf)
    w2b = w2.astype(bf)
    w3b = w3.astype(bf)
    in_maps = []
    for e in range(E):
        xg = x[tok[e]]                                    # [C, D]
        # [KC, 128, C] -> 4 chunks of 4 k-chunks, each packed
        # [128, n*C] with k-chunk-major columns
        xT = xg.T.astype(bf).reshape(KC, 128, C)

        def pack(lo, n):
            return np.ascontiguousarray(
                xT[lo:lo + n].transpose(1, 0, 2)).reshape(128, n * C)
        x4 = np.stack([pack(4 * g, 4) for g in range(4)])  # [4, 128, 4C]
        # w1/w3 [I, D] -> [ic, j, kc, p] -> [ic, p, kc, j]
        w1p = np.ascontiguousarray(
            w1b[e].reshape(IC, 128, KC, 128).transpose(0, 3, 2, 1)
        ).reshape(IC, 128, KC * 128)
        w3p = np.ascontiguousarray(
            w3b[e].reshape(IC, 128, KC, 128).transpose(0, 3, 2, 1)
        ).reshape(IC, 128, KC * 128)
        # w2 [D, I] -> [dc, j, kic, p] -> [dc, p, kic, j]
        w2p = np.ascontiguousarray(
            w2b[e].reshape(DC, 128, IC, 128).transpose(0, 3, 2, 1)
        ).reshape(DC, 128, IC * 128)
        in_maps.append({"x4": x4, "w1": w1p, "w3": w3p, "w2": w2p})

    results = _run_spmd(nc, in_maps)

    # ---- host scatter (each assignment gathers its expert's row) ----
    R = np.stack([np.asarray(results[e]["o"]).astype(np.float32)
                  for e in range(E)])                 # [E, D, C]
    pos = np.searchsorted(uk, keys)                   # row within expert block
    row = pos - off[flat]
    on_dev = row < ndev[flat]
    out_flat = np.empty((T * A, D), dtype=np.float32)
    out_flat[on_dev] = R[flat[on_dev], :, row[on_dev]]

    # spill (unique pairs beyond C per expert): host compute
    if not np.all(on_dev):
        sp = np.nonzero(~on_dev)[0]
        for i in sp:
            e, t = flat[i], flat_tok[i]
            xs = x[t]
            h1 = xs @ w1[e].T
            h3 = xs @ w3[e].T
            h = (h1 / (1.0 + np.exp(-h1))) * h3
            out_flat[i] = h @ w2[e].T
    return out_flat.reshape(T, A, D)
